# revision 40
# baseline (speedup 1.0000x reference)
"""Trainium2 Bass kernel for nn_BatchedBitNetFFN (BitNet b1.58 batched-expert FFN).

Math per expert (reference semantics reproduced exactly up to fp ulp noise):
  xq   = fake-quant(x): per-token int8 absmax  -> xq_int in [-127,127] (exact in bf16)
  wq   = ternary quant: per-expert scale mean|w| -> wq_int in {-1,0,1} (exact in fp8e4)
  gate = xq @ wq_g^T ; up = xq @ wq_u^T        (exact integer GEMMs, f32 PSUM accum)
  h    = sigmoid(gate*scale) * up
  hq   = fake-quant(h) per-token over F
  out  = (hq @ wq_d^T) * folded_scales

Sharding: expert-parallel, E=16 experts -> 2 experts on each of 8 NeuronCores,
no cross-core communication. Rounding uses the +/- 1.5*2^23 magic constant
(exact round-to-nearest-even, matching jnp.round).

v3 layout/schedule (HW-measured 1.48 ms/call vs 1.82 ms baseline):
  - quantized transposed weights stored as fp8e4 (ternary is exact in fp8),
    double-buffered across experts; matmuls run mixed bf16-stationary x
    fp8-moving (verified exact + full-rate on HW).
  - expert e+1's weight pass (DMA + abs-accum + quantize + xbar-transpose +
    bf16->fp8 convert) is emitted interleaved into expert e's main loop, so
    the PE never drains between experts; x-quant for the next c-group is
    staged one chunk per chunk the same way.
  - magic-round multiplies (x-quant, w-quant) run on the ACT engine
    (Copy with scale ptr + bias); DVE keeps reduces and the final sub/cast.
    All bulk ops stay off gpsimd (measured 10-22x slower than modeled).
  - per-fg partial absmax reduces keep the h-quant chain off the PE
    critical path; GEMM2 for chunk i is deferred PEND chunks.
"""

import numpy as np

E_FULL, C_FULL, D, F = 16, 4096, 768, 2048
NCORES = 8
EPC = E_FULL // NCORES  # experts per core
MAGIC = 12582912.0  # 1.5 * 2**23 -> exact RNE integer rounding via add/sub
PEND = 6  # GEMM2 deferral depth (chunks)

_cache = {}


def emit(tc, x_d, wg_d, wu_d, wd_d, out_d, epc, C):
    import concourse.mybir as mybir

    nc = tc.nc
    f32 = mybir.dt.float32
    bf16 = mybir.dt.bfloat16
    fp8 = mybir.dt.float8e4
    AX = mybir.AxisListType.X
    OP = mybir.AluOpType
    AF = mybir.ActivationFunctionType

    DT = D // 128   # 6  d-chunks
    FT = F // 128   # 16 f-chunks
    FG = F // 512   # 4  f-groups (N=512)
    CG = C // 512   # c-groups per expert
    CPG = 4         # c-chunks (of 128 tokens) per c-group

    from contextlib import ExitStack
    ctx = ExitStack()
    tc._emit_ctx = ctx  # keep pools alive until TileContext exit

    const_p = ctx.enter_context(tc.tile_pool(name="const", bufs=1))
    wld_p = ctx.enter_context(tc.tile_pool(name="wld", bufs=3))
    parts_p = ctx.enter_context(tc.tile_pool(name="parts", bufs=2))
    bcast_p = ctx.enter_context(tc.tile_pool(name="bcast", bufs=2))
    tmp_p = ctx.enter_context(tc.tile_pool(name="tmp", bufs=12))
    tok_p = ctx.enter_context(tc.tile_pool(name="tok", bufs=16))
    wqs_p = ctx.enter_context(tc.tile_pool(name="wqs", bufs=2))
    wqTs_p = ctx.enter_context(tc.tile_pool(name="wqTs", bufs=2))
    wg8_p = ctx.enter_context(tc.tile_pool(name="wg8", bufs=2))
    wu8_p = ctx.enter_context(tc.tile_pool(name="wu8", bufs=2))
    wd8_p = ctx.enter_context(tc.tile_pool(name="wd8", bufs=1))
    xld_p = ctx.enter_context(tc.tile_pool(name="xld", bufs=2))
    xqs_p = ctx.enter_context(tc.tile_pool(name="xqs", bufs=2))
    xqT_p = ctx.enter_context(tc.tile_pool(name="xqTp", bufs=2))
    sg_p = ctx.enter_context(tc.tile_pool(name="sgp", bufs=3))
    t3_p = ctx.enter_context(tc.tile_pool(name="t3p", bufs=2))
    hq_p = ctx.enter_context(tc.tile_pool(name="hqp", bufs=2))
    hqT_p = ctx.enter_context(tc.tile_pool(name="hqTp", bufs=PEND + 2))
    out_p = ctx.enter_context(tc.tile_pool(name="outp", bufs=2))

    gp_p = ctx.enter_context(tc.tile_pool(name="gpp", bufs=2, space="PSUM"))
    up_p = ctx.enter_context(tc.tile_pool(name="upp", bufs=3, space="PSUM"))
    o1_p = ctx.enter_context(tc.tile_pool(name="o1p", bufs=3, space="PSUM"))

    ones128 = const_p.tile([128, 128], f32, name="ones128")
    nc.vector.memset(ones128[:], 1.0)

    # per-expert state filled by prologue: (bcast, wg8, wu8, wd8)
    pro_state = {}

    # weight descriptors: (dram, n 128-row tiles, row length, load split)
    # w_down tiles are loaded in two 1024-wide halves to halve the wtb
    # staging footprint.
    wspecs = [(wg_d, FT, D, 1), (wu_d, FT, D, 1), (wd_d, DT, F, 2)]

    def prologue(e):
        """Generator: quantize expert e's three weights into fp8 transposed
        SBUF tensors. Yields between steps for interleaving."""
        parts = parts_p.tile([128, FT], f32, tag="parts", name=f"parts{e}")
        bcast = bcast_p.tile([128, 8], f32, tag="bcast", name=f"bcast{e}")
        # per-fgroup weight tiles so each GEMM f-group only depends on its
        # own quantize+convert step (ramp: GEMM1 starts after group 0)
        wg8 = [wg8_p.tile([128, FG, DT, 128], fp8, tag=f"wg8_{h}",
                          name=f"wg8_{e}_{h}") for h in range(FG)]
        wu8 = [wu8_p.tile([128, FG, DT, 128], fp8, tag=f"wu8_{h}",
                          name=f"wu8_{e}_{h}") for h in range(FG)]
        wd8a = wd8_p.tile([128, 4, FT, 128], fp8, tag="wd8a",
                          name=f"wd8a_{e}")
        wd8b = wd8_p.tile([128, 2, FT, 128], fp8, tag="wd8b",
                          name=f"wd8b_{e}")
        pro_state[e] = (bcast, wg8, wu8, (wd8a, wd8b))
        for widx, (w_d, ntile, fd, nsplit) in enumerate(wspecs):
            tag = "wta" if fd == D else "wtb"
            ld = fd // nsplit
            # pass 1: sw = clip(mean|w|, 1e-5); two loads per step
            for t in range(ntile * nsplit):
                row, col = t // nsplit, t % nsplit
                wt = wld_p.tile([128, ld], f32, tag=tag, name=tag, bufs=3)
                nc.sync.dma_start(wt[:], w_d[e, 128 * row:128 * (row + 1),
                                             ld * col:ld * (col + 1)])
                nc.scalar.activation(wt[:], wt[:], AF.Abs,
                                     accum_out=parts[:, t:t + 1])
                if t % 2 == 1:
                    yield
            colsum = tmp_p.tile([128, 1], f32, name="colsum")
            nc.vector.tensor_reduce(colsum[:], parts[:, 0:ntile * nsplit],
                                    axis=AX, op=OP.add)
            bc_ps = o1_p.tile([128, 1], f32, tag="o1", name="bc_ps")
            nc.tensor.matmul(bc_ps[:], ones128[:], colsum[:],
                             start=True, stop=True)
            sw = bcast[:, 2 * widx:2 * widx + 1]
            rsw = bcast[:, 2 * widx + 1:2 * widx + 2]
            nc.vector.tensor_scalar(sw, bc_ps[:], 1.0 / (F * D), 1e-5,
                                    OP.mult, OP.max)
            nc.vector.reciprocal(rsw, sw)
            yield
            # pass 2: ternary quantize -> bf16 -> xbar transpose -> fp8
            grp = 4 if fd == D else 1
            for h in range(ntile // grp):
                wq_stage = wqs_p.tile([128, grp, fd], bf16, name="wqs")
                for tt in range(grp * nsplit):
                    row = h * grp + tt // nsplit
                    col = tt % nsplit
                    wt = wld_p.tile([128, ld], f32, tag=tag, name=tag, bufs=3)
                    nc.sync.dma_start(wt[:], w_d[e, 128 * row:128 * (row + 1),
                                                 ld * col:ld * (col + 1)])
                    # v = rsw*w + MAGIC (RNE int round); clip to [M-1, M+1]
                    nc.scalar.activation(wt[:], wt[:], AF.Copy,
                                         bias=MAGIC, scale=rsw)
                    nc.vector.tensor_scalar(wt[:], wt[:], MAGIC - 1.0,
                                            MAGIC + 1.0, OP.max, OP.min)
                    nc.vector.tensor_scalar(
                        wq_stage[:, tt // nsplit, ld * col:ld * (col + 1)],
                        wt[:], MAGIC, None, OP.subtract)
                nblk = grp * fd // 128
                wqT_s = wqTs_p.tile([128, nblk, 128], bf16, name="wqTs")
                nc.sync.dma_start_transpose(wqT_s[:], wq_stage[:])
                if widx == 0:
                    dst = wg8[h][:]
                elif widx == 1:
                    dst = wu8[h][:]
                elif h < 4:
                    dst = wd8a[:, h:h + 1, :, :]
                else:
                    dst = wd8b[:, h - 4:h - 3, :, :]
                nc.vector.tensor_copy(dst, wqT_s[:])
                yield
        # kb = sw_u * sw_d / 127
        nc.vector.tensor_mul(bcast[:, 6:7], bcast[:, 2:3], bcast[:, 4:5])
        nc.vector.tensor_scalar_mul(bcast[:, 6:7], bcast[:, 6:7], 1.0 / 127.0)
        yield

    def advance(gen, n=None):
        if gen is None:
            return
        try:
            if n is None:
                while True:
                    next(gen)
            else:
                for _ in range(n):
                    next(gen)
        except StopIteration:
            pass

    # ---------- main loop pieces ----------
    def x_chunk(e, g, ct, xq_stage):
        """Load + act-quant one c-chunk of 128 tokens into xq_stage[:, ct]."""
        ci = g * CPG + ct
        xt = xld_p.tile([128, D], f32, name="xt")
        nc.sync.dma_start(xt[:], x_d[e, 128 * ci:128 * (ci + 1), :])
        amax = tmp_p.tile([128, 1], f32, name="amax")
        nc.vector.tensor_reduce(amax[:], xt[:], axis=AX, op=OP.max,
                                apply_absolute_value=True)
        inv_sx = tok_p.tile([128, 1], f32, tag="isx", name="inv_sx")
        nc.vector.tensor_scalar(inv_sx[:], amax[:], 1e-5, 1.0 / 127.0,
                                OP.max, OP.mult)
        sx = tmp_p.tile([128, 1], f32, name="sx")
        nc.vector.reciprocal(sx[:], inv_sx[:])
        nc.scalar.activation(xt[:], xt[:], AF.Copy, bias=MAGIC, scale=sx[:])
        nc.vector.tensor_scalar(xq_stage[:, ct, :], xt[:], MAGIC,
                                None, OP.subtract)
        return inv_sx

    def x_transpose(xq_stage):
        xqT = xqT_p.tile([128, CPG, DT, 128], bf16, name="xqT")
        nc.sync.dma_start_transpose(xqT[:], xq_stage[:])
        return xqT

    def x_stage(e, g):
        """Full c-group stage (4 chunks) in one go."""
        xq_stage = xqs_p.tile([128, CPG, D], bf16, name="xq_stage")
        inv_sxs = [x_chunk(e, g, ct, xq_stage) for ct in range(CPG)]
        return x_transpose(xq_stage), inv_sxs

    def gemm2_flush(e, pend_entry):
        bcast = pro_state[e][0]
        wd8a, wd8b = pro_state[e][3]
        hqT, inv_sx, m, ci = pend_entry
        o1 = o1_p.tile([128, 512], f32, tag="o1", name="o1")
        o2 = o1_p.tile([128, 256], f32, tag="o1", name="o2")
        for mi in range(FT):
            nc.tensor.matmul(o1[:], hqT[:, mi, :], wd8a[:, :, mi, :],
                             start=(mi == 0), stop=(mi == FT - 1))
            nc.tensor.matmul(o2[:], hqT[:, mi, :], wd8b[:, :, mi, :],
                             start=(mi == 0), stop=(mi == FT - 1))
        s_out = tmp_p.tile([128, 1], f32, name="s_out")
        nc.vector.tensor_mul(s_out[:], inv_sx[:], m[:])
        nc.vector.tensor_mul(s_out[:], s_out[:], bcast[:, 6:7])
        ot = out_p.tile([128, D], f32, name="ot")
        nc.scalar.mul(ot[:, 0:512], o1[:], s_out[:])
        nc.scalar.mul(ot[:, 512:768], o2[:], s_out[:])
        nc.sync.dma_start(out_d[e, 128 * ci:128 * (ci + 1), :], ot[:])

    def main(e, next_gen, staged, next_staged, pro_total):
        bcast, wg8, wu8, _ = pro_state[e]
        pend = []
        done = 0
        for g in range(CG):
            if g not in staged:
                staged[g] = x_stage(e, g)
            xqT, inv_sxs = staged.pop(g)
            # group g+1 (wrapping into the next expert) staged incrementally,
            # one chunk per chunk
            if g + 1 < CG:
                nxt = (e, g + 1) if g + 1 not in staged else None
            elif e + 1 < epc:
                nxt = (e + 1, 0)
            else:
                nxt = None
            if nxt is not None:
                nxt_stage = xqs_p.tile([128, CPG, D], bf16, name="xq_stage")
                nxt_inv = []
            for ct in range(CPG):
                ci = g * CPG + ct
                inv_sx = inv_sxs[ct]
                s_g = tmp_p.tile([128, 1], f32, name="s_g")
                nc.vector.tensor_mul(s_g[:], inv_sx[:], bcast[:, 0:1])
                t3 = t3_p.tile([128, F], f32, name="t3")
                partials = tmp_p.tile([128, FG], f32, tag="partials",
                                      name="partials", bufs=4)
                for fg in range(FG):
                    gp = gp_p.tile([128, 512], f32, name="gp")
                    for k in range(DT):
                        nc.tensor.matmul(gp[:], xqT[:, ct, k, :],
                                         wg8[fg][:, :, k, :],
                                         start=(k == 0), stop=(k == DT - 1))
                    sg = sg_p.tile([128, 512], f32, name="sg")
                    nc.scalar.activation(sg[:], gp[:], AF.Sigmoid,
                                         bias=0.0, scale=s_g[:])
                    up = up_p.tile([128, 512], f32, name="up")
                    for k in range(DT):
                        nc.tensor.matmul(up[:], xqT[:, ct, k, :],
                                         wu8[fg][:, :, k, :],
                                         start=(k == 0), stop=(k == DT - 1))
                    seg = t3[:, 512 * fg:512 * (fg + 1)]
                    nc.vector.tensor_mul(seg, sg[:], up[:])
                    nc.vector.tensor_reduce(partials[:, fg:fg + 1], seg,
                                            axis=AX, op=OP.max,
                                            apply_absolute_value=True)
                m = tok_p.tile([128, 1], f32, tag="m", name="m")
                nc.vector.tensor_reduce(m[:], partials[:], axis=AX, op=OP.max)
                nc.vector.tensor_scalar_max(m[:], m[:], 1e-30)
                s2 = tmp_p.tile([128, 1], f32, name="s2")
                nc.vector.reciprocal(s2[:], m[:])
                nc.vector.tensor_scalar_mul(s2[:], s2[:], 127.0)
                nc.vector.tensor_scalar(t3[:], t3[:], s2[:], MAGIC,
                                        OP.mult, OP.add)
                hq = hq_p.tile([128, F], bf16, name="hq")
                nc.vector.tensor_scalar(hq[:], t3[:], MAGIC, None,
                                        OP.subtract)
                hqT = hqT_p.tile([128, FT, 128], bf16, name="hqT")
                nc.sync.dma_start_transpose(hqT[:], hq[:])
                pend.append((hqT, inv_sx, m, ci))
                if len(pend) > PEND:
                    gemm2_flush(e, pend.pop(0))
                # stage one chunk of the c-group two ahead
                if nxt is not None:
                    ne, ng = nxt
                    nxt_inv.append(x_chunk(ne, ng, ct, nxt_stage))
                    if ct == CPG - 1:
                        tgt = staged if ne == e else next_staged
                        tgt[ng] = (x_transpose(nxt_stage), nxt_inv)
                # interleave next expert's weight prep
                chunk_idx = g * CPG + ct
                want = (pro_total * (chunk_idx + 1)) // (CG * CPG)
                if next_gen is not None and want > done:
                    advance(next_gen, want - done)
                    done = want
        advance(next_gen)
        while pend:
            gemm2_flush(e, pend.pop(0))

    # ---------- drive ----------
    # prologue(0): interleave x prefetch of the first two groups
    PRO_TOTAL = 2 * (FT // 2 + 1 + FT // 4) + (DT + 1 + DT) + 1
    # prologue(0) with x chunks of groups 0/1 injected into the w_gate/w_up
    # pass-1 windows, so xqT(g0) is ready the moment wg8 lands
    gen0 = prologue(0)
    staged0 = {}
    s0 = xqs_p.tile([128, CPG, D], bf16, name="xq_stage")
    inv0 = []
    s1 = None
    for yi in range(1, PRO_TOTAL + 1):
        advance(gen0, 1)
        if yi in (2, 4, 6, 8):
            inv0.append(x_chunk(0, 0, len(inv0), s0))
        elif yi == 9:
            staged0[0] = (x_transpose(s0), inv0)
            s1 = xqs_p.tile([128, CPG, D], bf16, name="xq_stage")
            inv1 = []
        elif yi in (15, 17, 19, 21):
            inv1.append(x_chunk(0, 1, len(inv1), s1))
        elif yi == 22:
            staged0[1] = (x_transpose(s1), inv1)
    advance(gen0)

    staged = staged0
    for e in range(epc):
        next_gen = prologue(e + 1) if e + 1 < epc else None
        next_staged = {}
        main(e, next_gen, staged, next_staged, PRO_TOTAL)
        staged = next_staged

    ctx.close()


def build(epc=EPC, C=C_FULL, num_devices=NCORES, loop_k=None):
    import concourse.mybir as mybir
    import concourse.tile as tile
    from concourse import bacc

    nc = bacc.Bacc("TRN2", target_bir_lowering=False, debug=False,
                   num_devices=num_devices)
    f32 = mybir.dt.float32
    x_d = nc.dram_tensor("x", [epc, C, D], f32, kind="ExternalInput").ap()
    wg_d = nc.dram_tensor("w_gate", [epc, F, D], f32, kind="ExternalInput").ap()
    wu_d = nc.dram_tensor("w_up", [epc, F, D], f32, kind="ExternalInput").ap()
    wd_d = nc.dram_tensor("w_down", [epc, D, F], f32, kind="ExternalInput").ap()
    out_d = nc.dram_tensor("out", [epc, C, D], f32, kind="ExternalOutput").ap()
    with tile.TileContext(nc) as tc:
        if loop_k is None:
            emit(tc, x_d, wg_d, wu_d, wd_d, out_d, epc, C)
        else:
            with tc.For_i(0, loop_k, 1):
                emit(tc, x_d, wg_d, wu_d, wd_d, out_d, epc, C)
    nc.compile()
    return nc


def kernel(x, w_gate, w_up, w_down, _trace=False):
    from concourse.bass_utils import run_bass_kernel_spmd

    key = "nc"
    if key not in _cache:
        _cache[key] = build()
    nc = _cache[key]

    in_maps = []
    for mcore in range(NCORES):
        sl = slice(mcore * EPC, (mcore + 1) * EPC)
        in_maps.append({
            "x": np.ascontiguousarray(x[sl], dtype=np.float32),
            "w_gate": np.ascontiguousarray(w_gate[sl], dtype=np.float32),
            "w_up": np.ascontiguousarray(w_up[sl], dtype=np.float32),
            "w_down": np.ascontiguousarray(w_down[sl], dtype=np.float32),
        })
    res = run_bass_kernel_spmd(nc, in_maps, core_ids=list(range(NCORES)),
                               trace=_trace)
    out = np.concatenate([res.results[m]["out"] for m in range(NCORES)], axis=0)
    if _trace:
        _cache["last_results"] = res
    return out.astype(np.float32, copy=False)
